# revision 6
# baseline (speedup 1.0000x reference)
"""GVAE (4x GATv2 + z@z^T decode) Trainium2 kernel, 8 NeuronCores.

Contract: kernel(**inputs) takes the FULL inputs of reference.setup_inputs()
and returns (adj_recon[10000,10000], mu[10000,32], log_var[10000,32]) fp32.

Sharding: nodes row-sharded 1250/core. Edges partitioned by destination,
sorted by dst, grouped per 128-dst block, padded so every core runs an
identical program (SPMD). Per-edge gather of source features via dma_gather
from a bf16 table in DRAM; segment-softmax uses the max-free formulation
out = (sum exp(s)*xl) / (sum exp(s)) (scores are O(6) for these inputs);
segment sums are one-hot matmuls on the PE accumulating in PSUM.
AllGather collectives share per-layer gather tables + z across cores.
"""
import sys

sys.path.insert(0, "/opt/trn_rl_repo")

import numpy as np
import ml_dtypes

N = 10000
E = 320000
DIN = 256
HID = 64
HEADS = 4
LAT = 32
NEG = 0.2
NCORES = 8
SHARD = N // NCORES          # 1250
NBLK = 10                    # 9 full 128-dst blocks + one of 98
TILE_E = 1024                # edges per gather tile (8 subtiles of 128)
SUBT = TILE_E // 128
PAD_D = 255.0                # d_in_block for padding lanes (matches nothing)

bf = ml_dtypes.bfloat16


def _prep_edges(edge_index):
    """Shard+sort edges by destination; pad to a per-block tile structure
    that is identical across all 8 cores. Returns (Tb, per-core arrays)."""
    ei = np.asarray(edge_index).astype(np.int64)
    loop = np.arange(N, dtype=np.int64)
    src = np.concatenate([ei[0], loop])
    dst = np.concatenate([ei[1], loop])

    per_core = []
    counts = np.zeros((NCORES, NBLK), np.int64)
    for r in range(NCORES):
        m = (dst >= r * SHARD) & (dst < (r + 1) * SHARD)
        s = src[m]
        d = dst[m] - r * SHARD
        # sort by (dst-block, src): block grouping for the segment matmuls,
        # src-ascending within the block for gather locality in HBM
        order = np.lexsort((s, d >> 7))
        s, d = s[order], d[order]
        blk = d >> 7
        counts[r] = np.bincount(blk, minlength=NBLK)
        per_core.append((s, d, blk))

    Tb = [int(np.ceil(counts[:, b].max() / TILE_E)) for b in range(NBLK)]
    Ep = sum(Tb) * TILE_E
    offs = np.cumsum([0] + [t * TILE_E for t in Tb])

    idx_arrs, d_arrs, dT_arrs = [], [], []
    for r in range(NCORES):
        s, d, blk = per_core[r]
        src_pad = np.zeros(Ep, np.int16)
        d_pad = np.full(Ep, PAD_D, np.float32)
        pos = 0
        for b in range(NBLK):
            c = int(counts[r, b])
            sb = s[pos:pos + c]
            db = d[pos:pos + c] - b * 128
            o = offs[b]
            src_pad[o:o + c] = sb.astype(np.int16)
            d_pad[o:o + c] = db.astype(np.float32)
            pos += c
        idx = np.zeros((16, Ep // 16), np.int16)
        j = np.arange(Ep)
        idx[j % 16, j // 16] = src_pad
        idx = np.tile(idx, (8, 1))
        dv = np.zeros((128, Ep // 128), np.float32)
        dv[j % 128, j // 128] = d_pad
        dT = np.broadcast_to(d_pad.astype(bf).reshape(1, Ep), (128, Ep))
        idx_arrs.append(idx)
        d_arrs.append(dv)
        dT_arrs.append(np.ascontiguousarray(dT))
    return Tb, Ep, idx_arrs, d_arrs, dT_arrs


def _build(Tb, Ep):
    import concourse.bass as bass
    import concourse.bacc as bacc
    import concourse.mybir as mybir
    import concourse.tile as tile

    f32 = mybir.dt.float32
    bf16 = mybir.dt.bfloat16
    i16 = mybir.dt.int16
    i32 = mybir.dt.int32
    EQ = mybir.AluOpType.is_equal
    ADD = mybir.AluOpType.add
    MULT = mybir.AluOpType.mult
    MAXOP = mybir.AluOpType.max
    MINOP = mybir.AluOpType.min
    AF = mybir.ActivationFunctionType
    X = mybir.AxisListType.X

    nc = bacc.Bacc("TRN2", num_devices=NCORES)

    # ---- IO ----
    xT_in = nc.dram_tensor("xT", [DIN, SHARD], f32, kind="ExternalInput")
    eps_in = nc.dram_tensor("eps", [SHARD, LAT], f32, kind="ExternalInput")
    idx_in = nc.dram_tensor("idx", [128, Ep // 16], i16, kind="ExternalInput")
    dv_in = nc.dram_tensor("dv", [128, Ep // 128], f32, kind="ExternalInput")
    dT_in = nc.dram_tensor("dT", [128, Ep], bf16, kind="ExternalInput")
    wl0_in = nc.dram_tensor("wl0", [DIN, 256], f32, kind="ExternalInput")
    wr0_in = nc.dram_tensor("wr0", [DIN, 256], f32, kind="ExternalInput")
    wl1_in = nc.dram_tensor("wl1", [256, 256], f32, kind="ExternalInput")
    wr1_in = nc.dram_tensor("wr1", [256, 256], f32, kind="ExternalInput")
    wml_in = nc.dram_tensor("wml", [256, 64], f32, kind="ExternalInput")
    wmr_in = nc.dram_tensor("wmr", [256, 64], f32, kind="ExternalInput")
    att0_in = nc.dram_tensor("att0", [128, 2048], bf16, kind="ExternalInput")
    att1_in = nc.dram_tensor("att1", [128, 2048], bf16, kind="ExternalInput")
    attm_in = nc.dram_tensor("attm", [128, 512], bf16, kind="ExternalInput")

    adj_out = nc.dram_tensor("adj", [SHARD, N], f32, kind="ExternalOutput")
    mu_out = nc.dram_tensor("mu", [SHARD, LAT], f32, kind="ExternalOutput")
    lv_out = nc.dram_tensor("lv", [SHARD, LAT], f32, kind="ExternalOutput")

    # ---- internal DRAM (collectives) ----
    ag0_in = nc.dram_tensor("ag0_in", [SHARD, 256], bf16)
    tab0 = nc.dram_tensor("tab0", [N, 256], bf16, addr_space="Shared")
    ag1_in = nc.dram_tensor("ag1_in", [SHARD, 256], bf16)
    tab1 = nc.dram_tensor("tab1", [N, 256], bf16, addr_space="Shared")
    agm_in = nc.dram_tensor("agm_in", [SHARD, 128], bf16)
    tabm = nc.dram_tensor("tabm", [N, 128], bf16, addr_space="Shared")
    agz_in = nc.dram_tensor("agz_in", [SHARD, LAT], bf16)
    tabz = nc.dram_tensor("tabz", [N, LAT], bf16, addr_space="Shared")
    RG = [list(range(NCORES))]

    with tile.TileContext(nc) as tc:
        with (
            tc.tile_pool(name="constp", bufs=1) as constp,
            tc.tile_pool(name="persist", bufs=1) as persist,
            tc.tile_pool(name="sbw", bufs=2) as sbw,
            tc.tile_pool(name="psp", bufs=2, space="PSUM") as psp,
        ):
            # ---- constants ----
            iota_i = constp.tile([128, 128], i32)
            nc.gpsimd.iota(iota_i[:], pattern=[[1, 128]], base=0, channel_multiplier=0)
            iota_bf = constp.tile([128, 128], bf16)
            nc.vector.tensor_copy(iota_bf[:], iota_i[:])
            icol_i = constp.tile([128, 1], i32)
            nc.gpsimd.iota(icol_i[:], pattern=[[0, 1]], base=0, channel_multiplier=1)
            icol_f = constp.tile([128, 1], f32)
            nc.vector.tensor_copy(icol_f[:], icol_i[:])
            ident_bf = constp.tile([128, 128], bf16)
            nc.vector.tensor_scalar(ident_bf[:], iota_bf[:], icol_f[:], None, EQ)
            iota_f32 = constp.tile([128, 128], f32)
            nc.vector.tensor_copy(iota_f32[:], iota_i[:])
            ident_f32 = constp.tile([128, 128], f32)
            nc.vector.tensor_scalar(ident_f32[:], iota_f32[:], icol_f[:], None, EQ)

            def load_w(dram, outc, name):
                t = constp.tile([128, 2, outc], f32, name=name)
                nc.sync.dma_start(t[:], dram[:].rearrange("(k p) o -> p k o", p=128))
                return t

            wl0 = load_w(wl0_in, 256, "wl0_sb")
            wr0 = load_w(wr0_in, 256, "wr0_sb")
            wl1 = load_w(wl1_in, 256, "wl1_sb")
            wr1 = load_w(wr1_in, 256, "wr1_sb")
            wml = load_w(wml_in, 64, "wml_sb")
            wmr = load_w(wmr_in, 64, "wmr_sb")

            att0_sb = constp.tile([128, 32, 64], bf16)
            nc.sync.dma_start(att0_sb[:], att0_in[:].rearrange("p (s c) -> p s c", s=32))
            att1_sb = constp.tile([128, 32, 64], bf16)
            nc.sync.dma_start(att1_sb[:], att1_in[:].rearrange("p (s c) -> p s c", s=32))
            attm_sb = constp.tile([128, 16, 32], bf16)
            nc.sync.dma_start(attm_sb[:], attm_in[:].rearrange("p (s c) -> p s c", s=16))

            idx_sb = constp.tile([128, Ep // 16], i16)
            nc.sync.dma_start(idx_sb[:], idx_in[:])
            d_sb = constp.tile([128, Ep // 128], f32)
            nc.sync.dma_start(d_sb[:], dv_in[:])

            ROWS = [128] * 9 + [98]

            # ---------------- node linear phase ----------------
            def node_phase(hT, w_l, w_r, outc_l, outc_r, ag_dram, xr_store, pad128):
                """xl rows -> bf16 -> ag_dram; xr rows -> xr_store [128, NBLK, outc_r]."""
                for nt in range(NBLK):
                    rows = ROWS[nt]
                    ps = psp.tile([128, outc_l], f32, tag="shared")
                    for k in range(2):
                        nc.tensor.matmul(ps[:rows], hT[:, k, nt * 128:nt * 128 + rows],
                                         w_l[:, k, :], start=(k == 0), stop=(k == 1))
                    cw = 128 if pad128 else outc_l
                    bfx = sbw.tile([128, cw], bf16, tag="xlbf", bufs=3)
                    if pad128:
                        nc.gpsimd.memset(bfx[:], 0.0)
                    nc.scalar.copy(bfx[:rows, 0:outc_l], ps[:rows])
                    nc.sync.dma_start(ag_dram[nt * 128:nt * 128 + rows, :], bfx[:rows])
                    ps2 = psp.tile([128, outc_r], f32, tag="shared")
                    for k in range(2):
                        nc.tensor.matmul(ps2[:rows], hT[:, k, nt * 128:nt * 128 + rows],
                                         w_r[:, k, :], start=(k == 0), stop=(k == 1))
                    nc.scalar.copy(xr_store[:rows, nt, :], ps2[:rows])

            # ---------------- edge phase ----------------
            gtile_base = np.cumsum([0] + [t for t in Tb])

            def edge_phase(tab, elem, ch, nh, hc, att_sb, xr_store, finalize):
                nsc = SUBT * nh          # scores per tile row
                numc = ch + nh
                for b in range(NBLK):
                    rows = ROWS[b]
                    num_ps = psp.tile([128, numc], f32, tag="num")
                    for t in range(Tb[b]):
                        g = int(gtile_base[b]) + t
                        xl_g = sbw.tile([128, SUBT, elem], bf16, tag="xlg", bufs=3)
                        nc.gpsimd.dma_gather(xl_g[:], tab[:], idx_sb[:, g * (TILE_E // 16):(g + 1) * (TILE_E // 16)],
                                             TILE_E, TILE_E, elem)
                        dt_t = sbw.tile([128, TILE_E], bf16, tag="dt", bufs=3)
                        nc.sync.dma_start(dt_t[:], dT_in[:, g * TILE_E:(g + 1) * TILE_E])
                        l_t = sbw.tile([128, SUBT, ch], bf16, tag="ltile", bufs=2)
                        ohs = []
                        for s in range(SUBT):
                            col = g * SUBT + s
                            oh = sbw.tile([128, 128], bf16, tag="oh", bufs=2 * SUBT + 2)
                            nc.vector.tensor_scalar(oh[:], iota_bf[:], d_sb[:, col:col + 1], None, EQ)
                            ohT = sbw.tile([128, 128], bf16, tag="ohTs", bufs=4)
                            nc.vector.tensor_scalar(ohT[:], dt_t[:, s * 128:(s + 1) * 128], icol_f[:], None, EQ)
                            u_ps = psp.tile([128, ch], f32, tag="u", bufs=3)
                            nc.tensor.matmul(u_ps[:], ohT[:], xr_store[:, b, :],
                                             start=True, stop=False)
                            nc.tensor.matmul(u_ps[:], ident_bf[:], xl_g[:, s, 0:ch],
                                             start=False, stop=True)
                            nc.scalar.activation(l_t[:, s, :], u_ps[:], AF.Prelu,
                                                 bias=0.0, scale=1.0, alpha=NEG)
                            ohs.append(oh)
                        prod = sbw.tile([128, nsc, hc], bf16, tag="prod", bufs=2)
                        nc.vector.tensor_mul(prod[:], l_t[:].rearrange("p s (h c) -> p (s h) c", h=nh),
                                             att_sb[:])
                        score = sbw.tile([128, nsc], f32, tag="score", bufs=2)
                        nc.vector.tensor_reduce(score[:], prod[:], axis=X, op=ADD)
                        wt = sbw.tile([128, nsc], bf16, tag="wt", bufs=2)
                        nc.scalar.activation(wt[:], score[:], AF.Exp)
                        wg = sbw.tile([128, SUBT, numc], bf16, tag="wg", bufs=2)
                        w_view = wt[:].rearrange("p (s h) -> p s h", s=SUBT).unsqueeze(-1) \
                                      .broadcast_to([128, SUBT, nh, hc])
                        nc.vector.tensor_mul(wg[:, :, 0:ch].rearrange("p s (h c) -> p s h c", h=nh),
                                             xl_g[:, :, 0:ch].rearrange("p s (h c) -> p s h c", h=nh),
                                             w_view)
                        nc.vector.tensor_copy(wg[:, :, ch:numc],
                                              wt[:].rearrange("p (s h) -> p s h", s=SUBT))
                        for s in range(SUBT):
                            nc.tensor.matmul(num_ps[:], ohs[s][:], wg[:, s, :],
                                             start=(t == 0 and s == 0),
                                             stop=(t == Tb[b] - 1 and s == SUBT - 1),
                                             skip_group_check=True)
                    finalize(b, rows, num_ps)

            # ---------------- per-layer finalize ----------------
            def make_helu_finalize(h_elu):
                def fin(b, rows, num_ps):
                    recip = sbw.tile([128, HEADS], f32, tag="recip", bufs=2)
                    nc.vector.reciprocal(recip[:rows], num_ps[:rows, 256:260])
                    r_t = sbw.tile([128, 256], f32, tag="relu_t", bufs=2)
                    m_t = sbw.tile([128, 256], f32, tag="min_t", bufs=2)
                    for h in range(HEADS):
                        sl = slice(h * 64, (h + 1) * 64)
                        nc.vector.tensor_scalar(r_t[:rows, sl], num_ps[:rows, sl],
                                                recip[:rows, h:h + 1], 0.0, MULT, MAXOP)
                        nc.vector.tensor_scalar(m_t[:rows, sl], num_ps[:rows, sl],
                                                recip[:rows, h:h + 1], 0.0, MULT, MINOP)
                    e_t = sbw.tile([128, 256], f32, tag="exp_t", bufs=2)
                    nc.scalar.activation(e_t[:rows], m_t[:rows], AF.Exp)
                    nc.vector.tensor_scalar(e_t[:rows], e_t[:rows], -1.0, None, ADD)
                    nc.vector.tensor_add(h_elu[:rows, b, :], r_t[:rows], e_t[:rows])
                return fin

            def mulv_finalize(b, rows, num_ps):
                recip = sbw.tile([128, 2], f32, tag="recip", bufs=2)
                nc.vector.reciprocal(recip[:rows], num_ps[:rows, 64:66])
                mu_sb = sbw.tile([128, LAT], f32, tag="mu_sb", bufs=2)
                lv_sb = sbw.tile([128, LAT], f32, tag="lv_sb", bufs=2)
                nc.vector.tensor_scalar(mu_sb[:rows], num_ps[:rows, 0:32],
                                        recip[:rows, 0:1], None, MULT)
                nc.vector.tensor_scalar(lv_sb[:rows], num_ps[:rows, 32:64],
                                        recip[:rows, 1:2], None, MULT)
                nc.sync.dma_start(mu_out[b * 128:b * 128 + rows, :], mu_sb[:rows])
                nc.sync.dma_start(lv_out[b * 128:b * 128 + rows, :], lv_sb[:rows])
                eps_sb = sbw.tile([128, LAT], f32, tag="eps_sb", bufs=2)
                nc.sync.dma_start(eps_sb[:rows], eps_in[b * 128:b * 128 + rows, :])
                t_e = sbw.tile([128, LAT], f32, tag="texp", bufs=2)
                nc.scalar.activation(t_e[:rows], lv_sb[:rows], AF.Exp, bias=0.0, scale=0.5)
                z_sb = sbw.tile([128, LAT], f32, tag="z_sb", bufs=2)
                nc.vector.tensor_mul(z_sb[:rows], t_e[:rows], eps_sb[:rows])
                nc.vector.tensor_add(z_sb[:rows], z_sb[:rows], mu_sb[:rows])
                zbf = sbw.tile([128, LAT], bf16, tag="zbf", bufs=2)
                nc.vector.tensor_copy(zbf[:rows], z_sb[:rows])
                nc.sync.dma_start(agz_in[b * 128:b * 128 + rows, :], zbf[:rows])
                # z_locT for the adj matmul lhsT
                zT_ps = psp.tile([32, 128], bf16, tag="shared")
                nc.tensor.transpose(zT_ps[:, 0:rows], zbf[0:rows, :], ident_bf[0:rows, 0:rows])
                nc.scalar.copy(zlocT[:, b * 128:b * 128 + rows], zT_ps[:, 0:rows])

            # ---------------- transposes h_elu -> hT ----------------
            def transpose_h(h_elu, hT):
                for nt in range(NBLK):
                    rows = ROWS[nt]
                    for k in range(2):
                        tps = psp.tile([128, 128], f32, tag="shared")
                        nc.tensor.transpose(tps[:], h_elu[:, nt, k * 128:(k + 1) * 128],
                                            ident_f32[:])
                        nc.scalar.copy(hT[:, k, nt * 128:nt * 128 + rows], tps[:, 0:rows])

            # ================ the network ================
            hT0 = persist.tile([128, 2, SHARD], f32, tag="hT0")
            nc.sync.dma_start(hT0[:], xT_in[:].rearrange("(k p) n -> p k n", p=128))

            xr0 = persist.tile([128, NBLK, 256], bf16, tag="xr01")
            nc.gpsimd.memset(xr0[:], 0.0)
            node_phase(hT0, wl0, wr0, 256, 256, ag0_in, xr0, pad128=False)
            nc.gpsimd.collective_compute("AllGather", mybir.AluOpType.bypass, RG,
                                         ins=[ag0_in[:]], outs=[tab0[:]])
            h_elu0 = persist.tile([128, NBLK, 256], f32, tag="helu0")
            nc.gpsimd.memset(h_elu0[:], 0.0)
            edge_phase(tab0, 256, 256, HEADS, 64, att0_sb, xr0, make_helu_finalize(h_elu0))

            hT1 = persist.tile([128, 2, SHARD], f32, tag="hT1")
            transpose_h(h_elu0, hT1)
            xr1 = persist.tile([128, NBLK, 256], bf16, tag="xr01")
            nc.gpsimd.memset(xr1[:], 0.0)
            node_phase(hT1, wl1, wr1, 256, 256, ag1_in, xr1, pad128=False)
            nc.gpsimd.collective_compute("AllGather", mybir.AluOpType.bypass, RG,
                                         ins=[ag1_in[:]], outs=[tab1[:]])
            h_elu1 = persist.tile([128, NBLK, 256], f32, tag="helu0")
            nc.gpsimd.memset(h_elu1[:], 0.0)
            edge_phase(tab1, 256, 256, HEADS, 64, att1_sb, xr1, make_helu_finalize(h_elu1))

            hT2 = persist.tile([128, 2, SHARD], f32, tag="hT0")
            transpose_h(h_elu1, hT2)
            xrm = persist.tile([128, NBLK, 64], bf16, tag="xrm")
            nc.gpsimd.memset(xrm[:], 0.0)
            node_phase(hT2, wml, wmr, 64, 64, agm_in, xrm, pad128=True)
            nc.gpsimd.collective_compute("AllGather", mybir.AluOpType.bypass, RG,
                                         ins=[agm_in[:]], outs=[tabm[:]])
            zlocT = persist.tile([32, 1280], bf16, tag="zlocT")
            edge_phase(tabm, 128, 64, 2, 32, attm_sb, xrm, mulv_finalize)

            nc.gpsimd.collective_compute("AllGather", mybir.AluOpType.bypass, RG,
                                         ins=[agz_in[:]], outs=[tabz[:]])

            # ---------------- adj = z @ z^T ----------------
            zt_sb = persist.tile([128, 79, LAT], bf16, tag="zt")
            nc.sync.dma_start(zt_sb[:, 0:78, :],
                              tabz[0:9984, :].rearrange("(t p) c -> p t c", p=128))
            nc.sync.dma_start(zt_sb[0:16, 78, :], tabz[9984:10000, :])
            zallT = persist.tile([32, 10112], bf16, tag="zallT")
            for t in range(79):
                rt = 128 if t < 78 else 16
                zps = psp.tile([32, 128], bf16, tag="shared")
                nc.tensor.transpose(zps[:, 0:rt], zt_sb[0:rt, t, :], ident_bf[0:rt, 0:rt])
                nc.scalar.copy(zallT[:, t * 128:t * 128 + rt], zps[:, 0:rt])

            NCHUNK = [512] * 19 + [272]
            for mt in range(NBLK):
                rows = ROWS[mt]
                for ck in range(20):
                    c0 = ck * 512
                    cw = NCHUNK[ck]
                    aps = psp.tile([128, 512], f32, tag="shared")
                    nc.tensor.matmul(aps[:rows, 0:cw], zlocT[:, mt * 128:mt * 128 + rows],
                                     zallT[:, c0:c0 + cw], start=True, stop=True)
                    ast = sbw.tile([128, 512], f32, tag="astage", bufs=4)
                    if ck % 2 == 0:
                        nc.scalar.copy(ast[:rows, 0:cw], aps[:rows, 0:cw])
                    else:
                        nc.vector.tensor_copy(ast[:rows, 0:cw], aps[:rows, 0:cw])
                    nc.sync.dma_start(adj_out[mt * 128:mt * 128 + rows, c0:c0 + cw],
                                      ast[:rows, 0:cw])

    nc.compile()
    return nc


_CACHE = {}


def kernel(**inputs):
    from concourse.bass_utils import run_bass_kernel_spmd

    x = np.asarray(inputs["x"], np.float32)
    eps = np.asarray(inputs["eps"], np.float32)
    Tb, Ep, idx_arrs, d_arrs, dT_arrs = _prep_edges(inputs["edge_index"])

    # biases are zero in this problem's setup; verify and refuse silently-wrong output
    for k in inputs:
        if k.startswith(("bl_", "br_", "bo_")):
            assert not np.any(np.asarray(inputs[k])), f"nonzero bias {k} unsupported"

    key = tuple(Tb)
    if key not in _CACHE:
        _CACHE[key] = _build(Tb, Ep)
    nc = _CACHE[key]

    att0 = np.tile(np.asarray(inputs["att_l0"], np.float32).reshape(1, 256), (128, 8)).astype(bf)
    att1 = np.tile(np.asarray(inputs["att_l1"], np.float32).reshape(1, 256), (128, 8)).astype(bf)
    attm_row = np.concatenate([np.asarray(inputs["att_mu"], np.float32).reshape(32),
                               np.asarray(inputs["att_lv"], np.float32).reshape(32)])
    attm = np.tile(attm_row.reshape(1, 64), (128, 8)).astype(bf)
    wml = np.concatenate([np.asarray(inputs["Wl_mu"], np.float32),
                          np.asarray(inputs["Wl_lv"], np.float32)], axis=1)
    wmr = np.concatenate([np.asarray(inputs["Wr_mu"], np.float32),
                          np.asarray(inputs["Wr_lv"], np.float32)], axis=1)

    shared = {
        "wl0": np.ascontiguousarray(inputs["Wl_l0"], np.float32),
        "wr0": np.ascontiguousarray(inputs["Wr_l0"], np.float32),
        "wl1": np.ascontiguousarray(inputs["Wl_l1"], np.float32),
        "wr1": np.ascontiguousarray(inputs["Wr_l1"], np.float32),
        "wml": np.ascontiguousarray(wml),
        "wmr": np.ascontiguousarray(wmr),
        "att0": att0, "att1": att1, "attm": attm,
    }
    in_maps = []
    for r in range(NCORES):
        m = dict(shared)
        m["xT"] = np.ascontiguousarray(x[r * SHARD:(r + 1) * SHARD].T)
        m["eps"] = np.ascontiguousarray(eps[r * SHARD:(r + 1) * SHARD])
        m["idx"] = idx_arrs[r]
        m["dv"] = d_arrs[r]
        m["dT"] = dT_arrs[r]
        in_maps.append(m)

    res = run_bass_kernel_spmd(nc, in_maps, list(range(NCORES)))
    adj = np.concatenate([res.results[r]["adj"] for r in range(NCORES)], axis=0)
    mu = np.concatenate([res.results[r]["mu"] for r in range(NCORES)], axis=0)
    lv = np.concatenate([res.results[r]["lv"] for r in range(NCORES)], axis=0)
    return adj, mu, lv


# revision 7
# speedup vs baseline: 1.3158x; 1.3158x over previous
"""GVAE (4x GATv2 + z@z^T decode) Trainium2 kernel, 8 NeuronCores.

Contract: kernel(**inputs) takes the FULL inputs of reference.setup_inputs()
and returns (adj_recon[10000,10000], mu[10000,32], log_var[10000,32]) fp32.

Sharding: nodes row-sharded 1250/core. Edges partitioned by destination,
sorted by (dst-block, src), grouped per 128-dst block, padded so every core
runs an identical program (SPMD). Per-edge gather of source features via
dma_gather from a bf16 table in DRAM; segment-softmax uses the max-free
formulation out = (sum exp(s)*xl)/(sum exp(s)) (scores are O(6) here);
segment sums are one-hot matmuls on the PE accumulating in PSUM.
AllGather collectives share per-layer gather tables + z across cores.
"""
import sys

sys.path.insert(0, "/opt/trn_rl_repo")

import numpy as np
import ml_dtypes

N = 10000
E = 320000
DIN = 256
HID = 64
HEADS = 4
LAT = 32
NEG = 0.2
NCORES = 8
SHARD = N // NCORES          # 1250
NBLK = 10                    # 9 full 128-dst blocks + one of 98
TILE_E = 1024                # max edges per gather tile
PAD_D = 255.0                # d_in_block for padding lanes (matches nothing)

bf = ml_dtypes.bfloat16


def _prep_edges(edge_index):
    """Shard+sort edges by destination; pad to a per-block tile structure
    identical across all 8 cores."""
    ei = np.asarray(edge_index).astype(np.int64)
    loop = np.arange(N, dtype=np.int64)
    src = np.concatenate([ei[0], loop])
    dst = np.concatenate([ei[1], loop])

    per_core = []
    counts = np.zeros((NCORES, NBLK), np.int64)
    for r in range(NCORES):
        m = (dst >= r * SHARD) & (dst < (r + 1) * SHARD)
        s = src[m]
        d = dst[m] - r * SHARD
        # sort by (dst-block, src): block grouping for the segment matmuls,
        # src-ascending within the block for gather locality
        order = np.lexsort((s, d >> 7))
        s, d = s[order], d[order]
        counts[r] = np.bincount(d >> 7, minlength=NBLK)
        per_core.append((s, d))

    # per-block padded edge count (multiple of 128) and tile plan
    Pb = [int(np.ceil(counts[:, b].max() / 128) * 128) for b in range(NBLK)]
    plan = []          # per block: list of tile sizes (multiples of 128, <=1024)
    for b in range(NBLK):
        rem = Pb[b]
        tiles = []
        while rem > 0:
            t = min(TILE_E, rem)
            tiles.append(t)
            rem -= t
        plan.append(tiles)
    Ep = sum(Pb)
    offs = np.cumsum([0] + Pb)

    idx_arrs, d_arrs, dT_arrs = [], [], []
    j = np.arange(Ep)
    for r in range(NCORES):
        s, d = per_core[r]
        src_pad = np.zeros(Ep, np.int16)
        d_pad = np.full(Ep, PAD_D, np.float32)
        pos = 0
        for b in range(NBLK):
            c = int(counts[r, b])
            o = int(offs[b])
            src_pad[o:o + c] = s[pos:pos + c].astype(np.int16)
            d_pad[o:o + c] = (d[pos:pos + c] - b * 128).astype(np.float32)
            pos += c
        idx = np.zeros((16, Ep // 16), np.int16)
        idx[j % 16, j // 16] = src_pad
        idx = np.tile(idx, (8, 1))
        dv = np.zeros((128, Ep // 128), bf)
        dv[j % 128, j // 128] = d_pad.astype(bf)
        dT = np.broadcast_to(d_pad.astype(bf).reshape(1, Ep), (128, Ep))
        idx_arrs.append(idx)
        d_arrs.append(dv)
        dT_arrs.append(np.ascontiguousarray(dT))
    return plan, Ep, offs, idx_arrs, d_arrs, dT_arrs


def _build(plan, Ep, offs):
    import concourse.bacc as bacc
    import concourse.mybir as mybir
    import concourse.tile as tile

    f32 = mybir.dt.float32
    bf16 = mybir.dt.bfloat16
    i16 = mybir.dt.int16
    i32 = mybir.dt.int32
    EQ = mybir.AluOpType.is_equal
    ADD = mybir.AluOpType.add
    MULT = mybir.AluOpType.mult
    MAXOP = mybir.AluOpType.max
    MINOP = mybir.AluOpType.min
    AF = mybir.ActivationFunctionType
    X = mybir.AxisListType.X

    nc = bacc.Bacc("TRN2", num_devices=NCORES)

    # ---- IO ----
    xT_in = nc.dram_tensor("xT", [DIN, SHARD], f32, kind="ExternalInput")
    eps_in = nc.dram_tensor("eps", [SHARD, LAT], f32, kind="ExternalInput")
    idx_in = nc.dram_tensor("idx", [128, Ep // 16], i16, kind="ExternalInput")
    dv_in = nc.dram_tensor("dv", [128, Ep // 128], bf16, kind="ExternalInput")
    dT_in = nc.dram_tensor("dT", [128, Ep], bf16, kind="ExternalInput")
    wl0_in = nc.dram_tensor("wl0", [DIN, 256], f32, kind="ExternalInput")
    wr0_in = nc.dram_tensor("wr0", [DIN, 256], f32, kind="ExternalInput")
    wl1_in = nc.dram_tensor("wl1", [256, 256], f32, kind="ExternalInput")
    wr1_in = nc.dram_tensor("wr1", [256, 256], f32, kind="ExternalInput")
    wml_in = nc.dram_tensor("wml", [256, 64], f32, kind="ExternalInput")
    wmr_in = nc.dram_tensor("wmr", [256, 64], f32, kind="ExternalInput")
    att0_in = nc.dram_tensor("att0", [128, 2048], bf16, kind="ExternalInput")
    att1_in = nc.dram_tensor("att1", [128, 2048], bf16, kind="ExternalInput")
    attm_in = nc.dram_tensor("attm", [128, 512], bf16, kind="ExternalInput")

    adj_out = nc.dram_tensor("adj", [SHARD, N], f32, kind="ExternalOutput")
    mu_out = nc.dram_tensor("mu", [SHARD, LAT], f32, kind="ExternalOutput")
    lv_out = nc.dram_tensor("lv", [SHARD, LAT], f32, kind="ExternalOutput")

    # ---- internal DRAM (collectives) ----
    ag0_in = nc.dram_tensor("ag0_in", [SHARD, 256], bf16)
    tab0 = nc.dram_tensor("tab0", [N, 256], bf16, addr_space="Shared")
    ag1_in = nc.dram_tensor("ag1_in", [SHARD, 256], bf16)
    tab1 = nc.dram_tensor("tab1", [N, 256], bf16, addr_space="Shared")
    agm_in = nc.dram_tensor("agm_in", [SHARD, 128], bf16)
    tabm = nc.dram_tensor("tabm", [N, 128], bf16, addr_space="Shared")
    agz_in = nc.dram_tensor("agz_in", [SHARD, LAT], bf16)
    tabz = nc.dram_tensor("tabz", [N, LAT], bf16, addr_space="Shared")
    RG = [list(range(NCORES))]

    with tile.TileContext(nc) as tc:
        with (
            tc.tile_pool(name="constp", bufs=1) as constp,
            tc.tile_pool(name="persist", bufs=1) as persist,
            tc.tile_pool(name="sbw", bufs=2) as sbw,
            tc.tile_pool(name="psp", bufs=2, space="PSUM") as psp,
        ):
            # ---- constants ----
            iota_i = constp.tile([128, 128], i32)
            nc.gpsimd.iota(iota_i[:], pattern=[[1, 128]], base=0, channel_multiplier=0)
            iota_bf = constp.tile([128, 128], bf16)
            nc.vector.tensor_copy(iota_bf[:], iota_i[:])
            icol_i = constp.tile([128, 1], i32)
            nc.gpsimd.iota(icol_i[:], pattern=[[0, 1]], base=0, channel_multiplier=1)
            icol_f = constp.tile([128, 1], f32)
            nc.vector.tensor_copy(icol_f[:], icol_i[:])
            icol_bf = constp.tile([128, 1], bf16)
            nc.vector.tensor_copy(icol_bf[:], icol_i[:])
            ident_bf = constp.tile([128, 128], bf16)
            nc.vector.tensor_scalar(ident_bf[:], iota_bf[:], icol_f[:], None, EQ)
            iota_f32 = constp.tile([128, 128], f32)
            nc.vector.tensor_copy(iota_f32[:], iota_i[:])
            ident_f32 = constp.tile([128, 128], f32)
            nc.vector.tensor_scalar(ident_f32[:], iota_f32[:], icol_f[:], None, EQ)

            def load_w(dram, outc, name):
                t = constp.tile([128, 2, outc], f32, name=name)
                nc.sync.dma_start(t[:], dram[:].rearrange("(k p) o -> p k o", p=128))
                return t

            wl0 = load_w(wl0_in, 256, "wl0_sb")
            wr0 = load_w(wr0_in, 256, "wr0_sb")
            wl1 = load_w(wl1_in, 256, "wl1_sb")
            wr1 = load_w(wr1_in, 256, "wr1_sb")
            wml = load_w(wml_in, 64, "wml_sb")
            wmr = load_w(wmr_in, 64, "wmr_sb")

            att0_sb = constp.tile([128, 32, 64], bf16)
            nc.sync.dma_start(att0_sb[:], att0_in[:].rearrange("p (s c) -> p s c", s=32))
            att1_sb = constp.tile([128, 32, 64], bf16)
            nc.sync.dma_start(att1_sb[:], att1_in[:].rearrange("p (s c) -> p s c", s=32))
            attm_sb = constp.tile([128, 16, 32], bf16)
            nc.sync.dma_start(attm_sb[:], attm_in[:].rearrange("p (s c) -> p s c", s=16))

            idx_sb = constp.tile([128, Ep // 16], i16)
            nc.sync.dma_start(idx_sb[:], idx_in[:])
            d_sb = constp.tile([128, Ep // 128], bf16)
            nc.sync.dma_start(d_sb[:], dv_in[:])

            ROWS = [128] * 9 + [98]

            # ---------------- node linear phase ----------------
            def node_phase(hT, w_l, w_r, outc_l, outc_r, ag_dram, xr_store, pad128):
                for nt in range(NBLK):
                    rows = ROWS[nt]
                    ps = psp.tile([128, outc_l], f32, tag="shared")
                    for k in range(2):
                        nc.tensor.matmul(ps[:rows], hT[:, k, nt * 128:nt * 128 + rows],
                                         w_l[:, k, :], start=(k == 0), stop=(k == 1))
                    cw = 128 if pad128 else outc_l
                    bfx = sbw.tile([128, cw], bf16, tag="xlbf", bufs=3)
                    if pad128:
                        nc.gpsimd.memset(bfx[:], 0.0)
                    nc.scalar.copy(bfx[:rows, 0:outc_l], ps[:rows])
                    nc.sync.dma_start(ag_dram[nt * 128:nt * 128 + rows, :], bfx[:rows])
                    ps2 = psp.tile([128, outc_r], f32, tag="shared")
                    for k in range(2):
                        nc.tensor.matmul(ps2[:rows], hT[:, k, nt * 128:nt * 128 + rows],
                                         w_r[:, k, :], start=(k == 0), stop=(k == 1))
                    nc.scalar.copy(xr_store[:rows, nt, :], ps2[:rows])

            # ---------------- edge phase ----------------
            def edge_phase(tab, elem, ch, nh, hc, att_sb, xr_store, finalize):
                numc = ch + nh
                for b in range(NBLK):
                    rows = ROWS[b]
                    num_ps = psp.tile([128, numc], f32, tag="num")
                    ntiles = len(plan[b])
                    off = int(offs[b])
                    for t, ts in enumerate(plan[b]):
                        nsub = ts // 128
                        xl_g = sbw.tile([128, TILE_E // 128, elem], bf16, tag="xlg", bufs=3)
                        nc.gpsimd.dma_gather(xl_g[:, 0:nsub, :], tab[:],
                                             idx_sb[:, off // 16:(off + ts) // 16],
                                             ts, ts, elem)
                        dt_t = sbw.tile([128, TILE_E], bf16, tag="dt", bufs=3)
                        nc.sync.dma_start(dt_t[:, 0:ts], dT_in[:, off:off + ts])
                        l_t = sbw.tile([128, TILE_E // 128, ch], bf16, tag="ltile", bufs=2)
                        ohs = []
                        for s in range(nsub):
                            col = off // 128 + s
                            oh = sbw.tile([128, 128], bf16, tag="oh", bufs=20)
                            nc.vector.tensor_tensor(oh[:], iota_bf[:],
                                                    d_sb[:, col:col + 1].broadcast_to([128, 128]), EQ)
                            ohT = sbw.tile([128, 128], bf16, tag="ohTs", bufs=4)
                            nc.vector.tensor_tensor(ohT[:], dt_t[:, s * 128:(s + 1) * 128],
                                                    icol_bf[:].broadcast_to([128, 128]), EQ)
                            u_ps = psp.tile([128, ch], f32, tag="u", bufs=3)
                            nc.tensor.matmul(u_ps[:], ohT[:], xr_store[:, b, :],
                                             start=True, stop=False)
                            nc.tensor.matmul(u_ps[:], ident_bf[:], xl_g[:, s, 0:ch],
                                             start=False, stop=True)
                            nc.scalar.activation(l_t[:, s, :], u_ps[:], AF.Prelu,
                                                 bias=0.0, scale=1.0, alpha=NEG)
                            ohs.append(oh)
                        nsc = nsub * nh
                        prod = sbw.tile([128, (TILE_E // 128) * nh, hc], bf16, tag="prod", bufs=2)
                        nc.vector.tensor_mul(prod[:, 0:nsc, :],
                                             l_t[:, 0:nsub, :].rearrange("p s (h c) -> p (s h) c", h=nh),
                                             att_sb[:, 0:nsc, :])
                        score = sbw.tile([128, (TILE_E // 128) * nh], f32, tag="score", bufs=2)
                        nc.vector.tensor_reduce(score[:, 0:nsc], prod[:, 0:nsc, :], axis=X, op=ADD)
                        wt = sbw.tile([128, (TILE_E // 128) * nh], bf16, tag="wt", bufs=2)
                        nc.scalar.activation(wt[:, 0:nsc], score[:, 0:nsc], AF.Exp)
                        wg = sbw.tile([128, TILE_E // 128, numc], bf16, tag="wg", bufs=2)
                        w_view = wt[:, 0:nsc].rearrange("p (s h) -> p s h", s=nsub).unsqueeze(-1) \
                                             .broadcast_to([128, nsub, nh, hc])
                        nc.vector.tensor_mul(wg[:, 0:nsub, 0:ch].rearrange("p s (h c) -> p s h c", h=nh),
                                             xl_g[:, 0:nsub, 0:ch].rearrange("p s (h c) -> p s h c", h=nh),
                                             w_view)
                        nc.vector.tensor_copy(wg[:, 0:nsub, ch:numc],
                                              wt[:, 0:nsc].rearrange("p (s h) -> p s h", s=nsub))
                        for s in range(nsub):
                            nc.tensor.matmul(num_ps[:], ohs[s][:], wg[:, s, :],
                                             start=(t == 0 and s == 0),
                                             stop=(t == ntiles - 1 and s == nsub - 1),
                                             skip_group_check=True)
                        off += ts
                    finalize(b, rows, num_ps)

            # ---------------- per-layer finalize ----------------
            def make_helu_finalize(h_elu):
                def fin(b, rows, num_ps):
                    recip = sbw.tile([128, HEADS], f32, tag="recip", bufs=2)
                    nc.vector.reciprocal(recip[:rows], num_ps[:rows, 256:260])
                    r_t = sbw.tile([128, 256], f32, tag="relu_t", bufs=2)
                    m_t = sbw.tile([128, 256], f32, tag="min_t", bufs=2)
                    for h in range(HEADS):
                        sl = slice(h * 64, (h + 1) * 64)
                        rb = recip[:rows, h:h + 1].broadcast_to([rows, 64])
                        nc.vector.scalar_tensor_tensor(r_t[:rows, sl], num_ps[:rows, sl],
                                                       0.0, rb, MAXOP, MULT)
                        nc.vector.scalar_tensor_tensor(m_t[:rows, sl], num_ps[:rows, sl],
                                                       0.0, rb, MINOP, MULT)
                    e_t = sbw.tile([128, 256], f32, tag="exp_t", bufs=2)
                    nc.scalar.activation(e_t[:rows], m_t[:rows], AF.Exp)
                    nc.vector.tensor_scalar(e_t[:rows], e_t[:rows], -1.0, None, ADD)
                    nc.vector.tensor_add(h_elu[:rows, b, :], r_t[:rows], e_t[:rows])
                return fin

            def mulv_finalize(b, rows, num_ps):
                recip = sbw.tile([128, 2], f32, tag="recip", bufs=2)
                nc.vector.reciprocal(recip[:rows], num_ps[:rows, 64:66])
                mu_sb = sbw.tile([128, LAT], f32, tag="mu_sb", bufs=2)
                lv_sb = sbw.tile([128, LAT], f32, tag="lv_sb", bufs=2)
                nc.vector.tensor_tensor(mu_sb[:rows], num_ps[:rows, 0:32],
                                        recip[:rows, 0:1].broadcast_to([rows, 32]), MULT)
                nc.vector.tensor_tensor(lv_sb[:rows], num_ps[:rows, 32:64],
                                        recip[:rows, 1:2].broadcast_to([rows, 32]), MULT)
                nc.sync.dma_start(mu_out[b * 128:b * 128 + rows, :], mu_sb[:rows])
                nc.sync.dma_start(lv_out[b * 128:b * 128 + rows, :], lv_sb[:rows])
                eps_sb = sbw.tile([128, LAT], f32, tag="eps_sb", bufs=2)
                nc.sync.dma_start(eps_sb[:rows], eps_in[b * 128:b * 128 + rows, :])
                t_e = sbw.tile([128, LAT], f32, tag="texp", bufs=2)
                nc.scalar.activation(t_e[:rows], lv_sb[:rows], AF.Exp, bias=0.0, scale=0.5)
                z_sb = sbw.tile([128, LAT], f32, tag="z_sb", bufs=2)
                nc.vector.tensor_mul(z_sb[:rows], t_e[:rows], eps_sb[:rows])
                nc.vector.tensor_add(z_sb[:rows], z_sb[:rows], mu_sb[:rows])
                zbf = sbw.tile([128, LAT], bf16, tag="zbf", bufs=2)
                nc.vector.tensor_copy(zbf[:rows], z_sb[:rows])
                nc.sync.dma_start(agz_in[b * 128:b * 128 + rows, :], zbf[:rows])
                zT_ps = psp.tile([32, 128], bf16, tag="shared")
                nc.tensor.transpose(zT_ps[:, 0:rows], zbf[0:rows, :], ident_bf[0:rows, 0:rows])
                nc.scalar.copy(zlocT[:, b * 128:b * 128 + rows], zT_ps[:, 0:rows])

            # ---------------- transposes h_elu -> hT ----------------
            def transpose_h(h_elu, hT):
                for nt in range(NBLK):
                    rows = ROWS[nt]
                    for k in range(2):
                        tps = psp.tile([128, 128], f32, tag="shared")
                        nc.tensor.transpose(tps[:], h_elu[:, nt, k * 128:(k + 1) * 128],
                                            ident_f32[:])
                        nc.scalar.copy(hT[:, k, nt * 128:nt * 128 + rows], tps[:, 0:rows])

            # ================ the network ================
            hT0 = persist.tile([128, 2, SHARD], f32, tag="hT0")
            nc.sync.dma_start(hT0[:], xT_in[:].rearrange("(k p) n -> p k n", p=128))

            xr0 = persist.tile([128, NBLK, 256], bf16, tag="xr01")
            nc.gpsimd.memset(xr0[:], 0.0)
            node_phase(hT0, wl0, wr0, 256, 256, ag0_in, xr0, pad128=False)
            nc.gpsimd.collective_compute("AllGather", mybir.AluOpType.bypass, RG,
                                         ins=[ag0_in[:]], outs=[tab0[:]])
            h_elu0 = persist.tile([128, NBLK, 256], f32, tag="helu0")
            nc.gpsimd.memset(h_elu0[:], 0.0)
            edge_phase(tab0, 256, 256, HEADS, 64, att0_sb, xr0, make_helu_finalize(h_elu0))

            hT1 = persist.tile([128, 2, SHARD], f32, tag="hT1")
            transpose_h(h_elu0, hT1)
            xr1 = persist.tile([128, NBLK, 256], bf16, tag="xr01")
            nc.gpsimd.memset(xr1[:], 0.0)
            node_phase(hT1, wl1, wr1, 256, 256, ag1_in, xr1, pad128=False)
            nc.gpsimd.collective_compute("AllGather", mybir.AluOpType.bypass, RG,
                                         ins=[ag1_in[:]], outs=[tab1[:]])
            h_elu1 = persist.tile([128, NBLK, 256], f32, tag="helu0")
            nc.gpsimd.memset(h_elu1[:], 0.0)
            edge_phase(tab1, 256, 256, HEADS, 64, att1_sb, xr1, make_helu_finalize(h_elu1))

            hT2 = persist.tile([128, 2, SHARD], f32, tag="hT0")
            transpose_h(h_elu1, hT2)
            xrm = persist.tile([128, NBLK, 64], bf16, tag="xrm")
            nc.gpsimd.memset(xrm[:], 0.0)
            node_phase(hT2, wml, wmr, 64, 64, agm_in, xrm, pad128=True)
            nc.gpsimd.collective_compute("AllGather", mybir.AluOpType.bypass, RG,
                                         ins=[agm_in[:]], outs=[tabm[:]])
            zlocT = persist.tile([32, 1280], bf16, tag="zlocT")
            edge_phase(tabm, 128, 64, 2, 32, attm_sb, xrm, mulv_finalize)

            nc.gpsimd.collective_compute("AllGather", mybir.AluOpType.bypass, RG,
                                         ins=[agz_in[:]], outs=[tabz[:]])

            # ---------------- adj = z @ z^T ----------------
            zt_sb = persist.tile([128, 79, LAT], bf16, tag="zt")
            nc.sync.dma_start(zt_sb[:, 0:78, :],
                              tabz[0:9984, :].rearrange("(t p) c -> p t c", p=128))
            nc.sync.dma_start(zt_sb[0:16, 78, :], tabz[9984:10000, :])
            zallT = persist.tile([32, 10112], bf16, tag="zallT")
            for t in range(79):
                rt = 128 if t < 78 else 16
                zps = psp.tile([32, 128], bf16, tag="shared")
                nc.tensor.transpose(zps[:, 0:rt], zt_sb[0:rt, t, :], ident_bf[0:rt, 0:rt])
                nc.scalar.copy(zallT[:, t * 128:t * 128 + rt], zps[:, 0:rt])

            NCHUNK = [512] * 19 + [272]
            for mt in range(NBLK):
                rows = ROWS[mt]
                for ck in range(20):
                    c0 = ck * 512
                    cw = NCHUNK[ck]
                    aps = psp.tile([128, 512], f32, tag="shared")
                    nc.tensor.matmul(aps[:rows, 0:cw], zlocT[:, mt * 128:mt * 128 + rows],
                                     zallT[:, c0:c0 + cw], start=True, stop=True)
                    ast = sbw.tile([128, 512], f32, tag="astage", bufs=4)
                    if ck % 2 == 0:
                        nc.scalar.copy(ast[:rows, 0:cw], aps[:rows, 0:cw])
                    else:
                        nc.vector.tensor_copy(ast[:rows, 0:cw], aps[:rows, 0:cw])
                    nc.sync.dma_start(adj_out[mt * 128:mt * 128 + rows, c0:c0 + cw],
                                      ast[:rows, 0:cw])

    nc.compile()
    return nc


_CACHE = {}


def kernel(**inputs):
    from concourse.bass_utils import run_bass_kernel_spmd

    x = np.asarray(inputs["x"], np.float32)
    eps = np.asarray(inputs["eps"], np.float32)
    plan, Ep, offs, idx_arrs, d_arrs, dT_arrs = _prep_edges(inputs["edge_index"])

    for k in inputs:
        if k.startswith(("bl_", "br_", "bo_")):
            assert not np.any(np.asarray(inputs[k])), f"nonzero bias {k} unsupported"

    key = tuple(tuple(p) for p in plan)
    if key not in _CACHE:
        _CACHE[key] = _build(plan, Ep, offs)
    nc = _CACHE[key]

    att0 = np.tile(np.asarray(inputs["att_l0"], np.float32).reshape(1, 256), (128, 8)).astype(bf)
    att1 = np.tile(np.asarray(inputs["att_l1"], np.float32).reshape(1, 256), (128, 8)).astype(bf)
    attm_row = np.concatenate([np.asarray(inputs["att_mu"], np.float32).reshape(32),
                               np.asarray(inputs["att_lv"], np.float32).reshape(32)])
    attm = np.tile(attm_row.reshape(1, 64), (128, 8)).astype(bf)
    wml = np.concatenate([np.asarray(inputs["Wl_mu"], np.float32),
                          np.asarray(inputs["Wl_lv"], np.float32)], axis=1)
    wmr = np.concatenate([np.asarray(inputs["Wr_mu"], np.float32),
                          np.asarray(inputs["Wr_lv"], np.float32)], axis=1)

    shared = {
        "wl0": np.ascontiguousarray(inputs["Wl_l0"], np.float32),
        "wr0": np.ascontiguousarray(inputs["Wr_l0"], np.float32),
        "wl1": np.ascontiguousarray(inputs["Wl_l1"], np.float32),
        "wr1": np.ascontiguousarray(inputs["Wr_l1"], np.float32),
        "wml": np.ascontiguousarray(wml),
        "wmr": np.ascontiguousarray(wmr),
        "att0": att0, "att1": att1, "attm": attm,
    }
    in_maps = []
    for r in range(NCORES):
        m = dict(shared)
        m["xT"] = np.ascontiguousarray(x[r * SHARD:(r + 1) * SHARD].T)
        m["eps"] = np.ascontiguousarray(eps[r * SHARD:(r + 1) * SHARD])
        m["idx"] = idx_arrs[r]
        m["dv"] = d_arrs[r]
        m["dT"] = dT_arrs[r]
        in_maps.append(m)

    res = run_bass_kernel_spmd(nc, in_maps, list(range(NCORES)))
    adj = np.concatenate([res.results[r]["adj"] for r in range(NCORES)], axis=0)
    mu = np.concatenate([res.results[r]["mu"] for r in range(NCORES)], axis=0)
    lv = np.concatenate([res.results[r]["lv"] for r in range(NCORES)], axis=0)
    return adj, mu, lv


# revision 8
# speedup vs baseline: 1.3810x; 1.0495x over previous
"""GVAE (4x GATv2 + z@z^T decode) Trainium2 kernel, 8 NeuronCores.

Contract: kernel(**inputs) takes the FULL inputs of reference.setup_inputs()
and returns (adj_recon[10000,10000], mu[10000,32], log_var[10000,32]) fp32.

Sharding: nodes row-sharded 1250/core. Edges partitioned by destination,
sorted by (dst-block, src), grouped per 128-dst block, padded so every core
runs an identical program (SPMD). Per-edge gather of source features via
dma_gather from a bf16 table in DRAM; segment-softmax uses the max-free
formulation out = (sum exp(s)*xl)/(sum exp(s)) (scores are O(6) here);
segment sums are one-hot matmuls on the PE accumulating in PSUM.
AllGather collectives share per-layer gather tables + z across cores.
"""
import sys

sys.path.insert(0, "/opt/trn_rl_repo")

import numpy as np
import ml_dtypes

N = 10000
E = 320000
DIN = 256
HID = 64
HEADS = 4
LAT = 32
NEG = 0.2
NCORES = 8
SHARD = N // NCORES          # 1250
NBLK = 10                    # 9 full 128-dst blocks + one of 98
TILE_E = 1024                # max edges per gather tile
PAD_D = 255.0                # d_in_block for padding lanes (matches nothing)

bf = ml_dtypes.bfloat16


def _prep_edges(edge_index):
    """Shard+sort edges by destination; pad to a per-block tile structure
    identical across all 8 cores."""
    ei = np.asarray(edge_index).astype(np.int64)
    loop = np.arange(N, dtype=np.int64)
    src = np.concatenate([ei[0], loop])
    dst = np.concatenate([ei[1], loop])

    per_core = []
    counts = np.zeros((NCORES, NBLK), np.int64)
    for r in range(NCORES):
        m = (dst >= r * SHARD) & (dst < (r + 1) * SHARD)
        s = src[m]
        d = dst[m] - r * SHARD
        # sort by (dst-block, src): block grouping for the segment matmuls,
        # src-ascending within the block for gather locality
        order = np.lexsort((s, d >> 7))
        s, d = s[order], d[order]
        counts[r] = np.bincount(d >> 7, minlength=NBLK)
        per_core.append((s, d))

    # per-block padded edge count (multiple of 128) and tile plan
    Pb = [int(np.ceil(counts[:, b].max() / 128) * 128) for b in range(NBLK)]
    plan = []          # per block: list of tile sizes (multiples of 128, <=1024)
    for b in range(NBLK):
        rem = Pb[b]
        tiles = []
        while rem > 0:
            t = min(TILE_E, rem)
            tiles.append(t)
            rem -= t
        plan.append(tiles)
    Ep = sum(Pb)
    offs = np.cumsum([0] + Pb)

    idx_arrs, d_arrs, dT_arrs = [], [], []
    j = np.arange(Ep)
    for r in range(NCORES):
        s, d = per_core[r]
        src_pad = np.zeros(Ep, np.int16)
        d_pad = np.full(Ep, PAD_D, np.float32)
        pos = 0
        for b in range(NBLK):
            c = int(counts[r, b])
            o = int(offs[b])
            src_pad[o:o + c] = s[pos:pos + c].astype(np.int16)
            d_pad[o:o + c] = (d[pos:pos + c] - b * 128).astype(np.float32)
            pos += c
        idx = np.zeros((16, Ep // 16), np.int16)
        idx[j % 16, j // 16] = src_pad
        idx = np.tile(idx, (8, 1))
        dv = np.zeros((128, Ep // 128), bf)
        dv[j % 128, j // 128] = d_pad.astype(bf)
        dT = np.broadcast_to(d_pad.astype(bf).reshape(1, Ep), (128, Ep))
        idx_arrs.append(idx)
        d_arrs.append(dv)
        dT_arrs.append(np.ascontiguousarray(dT))
    return plan, Ep, offs, idx_arrs, d_arrs, dT_arrs


def _build(plan, Ep, offs):
    import concourse.bacc as bacc
    import concourse.mybir as mybir
    import concourse.tile as tile

    f32 = mybir.dt.float32
    bf16 = mybir.dt.bfloat16
    i16 = mybir.dt.int16
    i32 = mybir.dt.int32
    EQ = mybir.AluOpType.is_equal
    ADD = mybir.AluOpType.add
    MULT = mybir.AluOpType.mult
    MAXOP = mybir.AluOpType.max
    MINOP = mybir.AluOpType.min
    AF = mybir.ActivationFunctionType
    X = mybir.AxisListType.X

    nc = bacc.Bacc("TRN2", num_devices=NCORES)

    # ---- IO ----
    xT_in = nc.dram_tensor("xT", [DIN, SHARD], f32, kind="ExternalInput")
    eps_in = nc.dram_tensor("eps", [SHARD, LAT], f32, kind="ExternalInput")
    idx_in = nc.dram_tensor("idx", [128, Ep // 16], i16, kind="ExternalInput")
    dv_in = nc.dram_tensor("dv", [128, Ep // 128], bf16, kind="ExternalInput")
    dT_in = nc.dram_tensor("dT", [128, Ep], bf16, kind="ExternalInput")
    wl0_in = nc.dram_tensor("wl0", [DIN, 256], f32, kind="ExternalInput")
    wr0_in = nc.dram_tensor("wr0", [DIN, 256], f32, kind="ExternalInput")
    wl1_in = nc.dram_tensor("wl1", [256, 256], f32, kind="ExternalInput")
    wr1_in = nc.dram_tensor("wr1", [256, 256], f32, kind="ExternalInput")
    wml_in = nc.dram_tensor("wml", [256, 64], f32, kind="ExternalInput")
    wmr_in = nc.dram_tensor("wmr", [256, 64], f32, kind="ExternalInput")
    att0_in = nc.dram_tensor("att0", [128, 2048], bf16, kind="ExternalInput")
    att1_in = nc.dram_tensor("att1", [128, 2048], bf16, kind="ExternalInput")
    attm_in = nc.dram_tensor("attm", [128, 512], bf16, kind="ExternalInput")

    adj_out = nc.dram_tensor("adj", [SHARD, N], f32, kind="ExternalOutput")
    mu_out = nc.dram_tensor("mu", [SHARD, LAT], f32, kind="ExternalOutput")
    lv_out = nc.dram_tensor("lv", [SHARD, LAT], f32, kind="ExternalOutput")

    # ---- internal DRAM (collectives) ----
    ag0_in = nc.dram_tensor("ag0_in", [SHARD, 256], bf16)
    tab0 = nc.dram_tensor("tab0", [N, 256], bf16, addr_space="Shared")
    ag1_in = nc.dram_tensor("ag1_in", [SHARD, 256], bf16)
    tab1 = nc.dram_tensor("tab1", [N, 256], bf16, addr_space="Shared")
    agm_in = nc.dram_tensor("agm_in", [SHARD, 128], bf16)
    tabm = nc.dram_tensor("tabm", [N, 128], bf16, addr_space="Shared")
    agz_in = nc.dram_tensor("agz_in", [SHARD, LAT], bf16)
    tabz = nc.dram_tensor("tabz", [N, LAT], bf16, addr_space="Shared")
    RG = [list(range(NCORES))]

    with tile.TileContext(nc) as tc:
        with (
            tc.tile_pool(name="constp", bufs=1) as constp,
            tc.tile_pool(name="persist", bufs=1) as persist,
            tc.tile_pool(name="sbw", bufs=2) as sbw,
            tc.tile_pool(name="psp", bufs=2, space="PSUM") as psp,
        ):
            # ---- constants ----
            iota_i = constp.tile([128, 128], i32)
            nc.gpsimd.iota(iota_i[:], pattern=[[1, 128]], base=0, channel_multiplier=0)
            iota_bf = constp.tile([128, 128], bf16)
            nc.vector.tensor_copy(iota_bf[:], iota_i[:])
            icol_i = constp.tile([128, 1], i32)
            nc.gpsimd.iota(icol_i[:], pattern=[[0, 1]], base=0, channel_multiplier=1)
            icol_f = constp.tile([128, 1], f32)
            nc.vector.tensor_copy(icol_f[:], icol_i[:])
            icol_bf = constp.tile([128, 1], bf16)
            nc.vector.tensor_copy(icol_bf[:], icol_i[:])
            ident_bf = constp.tile([128, 128], bf16)
            nc.vector.tensor_scalar(ident_bf[:], iota_bf[:], icol_f[:], None, EQ)
            iota_f32 = constp.tile([128, 128], f32)
            nc.vector.tensor_copy(iota_f32[:], iota_i[:])
            ident_f32 = constp.tile([128, 128], f32)
            nc.vector.tensor_scalar(ident_f32[:], iota_f32[:], icol_f[:], None, EQ)

            def load_w(dram, outc, name):
                t = constp.tile([128, 2, outc], f32, name=name)
                nc.sync.dma_start(t[:], dram[:].rearrange("(k p) o -> p k o", p=128))
                return t

            wl0 = load_w(wl0_in, 256, "wl0_sb")
            wr0 = load_w(wr0_in, 256, "wr0_sb")
            wl1 = load_w(wl1_in, 256, "wl1_sb")
            wr1 = load_w(wr1_in, 256, "wr1_sb")
            wml = load_w(wml_in, 64, "wml_sb")
            wmr = load_w(wmr_in, 64, "wmr_sb")

            att0_sb = constp.tile([128, 32, 64], bf16)
            nc.sync.dma_start(att0_sb[:], att0_in[:].rearrange("p (s c) -> p s c", s=32))
            att1_sb = constp.tile([128, 32, 64], bf16)
            nc.sync.dma_start(att1_sb[:], att1_in[:].rearrange("p (s c) -> p s c", s=32))
            attm_sb = constp.tile([128, 16, 32], bf16)
            nc.sync.dma_start(attm_sb[:], attm_in[:].rearrange("p (s c) -> p s c", s=16))

            idx_sb = constp.tile([128, Ep // 16], i16)
            nc.sync.dma_start(idx_sb[:], idx_in[:])
            d_sb = constp.tile([128, Ep // 128], bf16)
            nc.sync.dma_start(d_sb[:], dv_in[:])

            ROWS = [128] * 9 + [98]

            # ---------------- node linear phase ----------------
            def node_phase(hT, w_l, w_r, outc_l, outc_r, ag_dram, xr_store, pad128):
                for nt in range(NBLK):
                    rows = ROWS[nt]
                    ps = psp.tile([128, outc_l], f32, tag="shared")
                    for k in range(2):
                        nc.tensor.matmul(ps[:rows], hT[:, k, nt * 128:nt * 128 + rows],
                                         w_l[:, k, :], start=(k == 0), stop=(k == 1))
                    cw = 128 if pad128 else outc_l
                    bfx = sbw.tile([128, cw], bf16, tag="xlbf", bufs=3)
                    if pad128:
                        nc.gpsimd.memset(bfx[:], 0.0)
                    nc.scalar.copy(bfx[:rows, 0:outc_l], ps[:rows])
                    nc.sync.dma_start(ag_dram[nt * 128:nt * 128 + rows, :], bfx[:rows])
                    ps2 = psp.tile([128, outc_r], f32, tag="shared")
                    for k in range(2):
                        nc.tensor.matmul(ps2[:rows], hT[:, k, nt * 128:nt * 128 + rows],
                                         w_r[:, k, :], start=(k == 0), stop=(k == 1))
                    nc.scalar.copy(xr_store[:rows, nt, :], ps2[:rows])

            # ---------------- edge phase ----------------
            def edge_phase(tab, elem, ch, nh, hc, att_sb, xr_store, finalize):
                numc = ch + nh
                for b in range(NBLK):
                    rows = ROWS[b]
                    num_ps = psp.tile([128, numc], f32, tag="num")
                    ntiles = len(plan[b])
                    off = int(offs[b])
                    for t, ts in enumerate(plan[b]):
                        nsub = ts // 128
                        xl_g = sbw.tile([128, TILE_E // 128, elem], bf16, tag="xlg", bufs=3)
                        nc.gpsimd.dma_gather(xl_g[:, 0:nsub, :], tab[:],
                                             idx_sb[:, off // 16:(off + ts) // 16],
                                             ts, ts, elem)
                        dt_t = sbw.tile([128, TILE_E], bf16, tag="dt", bufs=3)
                        nc.sync.dma_start(dt_t[:, 0:ts], dT_in[:, off:off + ts])
                        l_t = sbw.tile([128, TILE_E // 128, ch], bf16, tag="ltile", bufs=2)
                        c0 = off // 128
                        oh_t = sbw.tile([128, TILE_E // 128, 128], bf16, tag="oh", bufs=3)
                        nc.vector.tensor_tensor(
                            oh_t[:, 0:nsub, :],
                            iota_bf[:].unsqueeze(1).broadcast_to([128, nsub, 128]),
                            d_sb[:, c0:c0 + nsub].unsqueeze(-1).broadcast_to([128, nsub, 128]), EQ)
                        ohT_t = sbw.tile([128, TILE_E // 128, 128], bf16, tag="ohTs", bufs=3)
                        nc.vector.tensor_tensor(
                            ohT_t[:, 0:nsub, :],
                            dt_t[:, 0:ts].rearrange("p (s e) -> p s e", e=128),
                            icol_bf[:].unsqueeze(-1).broadcast_to([128, nsub, 128]), EQ)
                        for s in range(nsub):
                            u_ps = psp.tile([128, ch], f32, tag="u", bufs=3)
                            nc.tensor.matmul(u_ps[:], ohT_t[:, s, :], xr_store[:, b, :],
                                             start=True, stop=False)
                            nc.tensor.matmul(u_ps[:], ident_bf[:], xl_g[:, s, 0:ch],
                                             start=False, stop=True)
                            nc.scalar.activation(l_t[:, s, :], u_ps[:], AF.Prelu,
                                                 bias=0.0, scale=1.0, alpha=NEG)
                        nsc = nsub * nh
                        prod = sbw.tile([128, (TILE_E // 128) * nh, hc], bf16, tag="prod", bufs=2)
                        nc.vector.tensor_mul(prod[:, 0:nsc, :],
                                             l_t[:, 0:nsub, :].rearrange("p s (h c) -> p (s h) c", h=nh),
                                             att_sb[:, 0:nsc, :])
                        score = sbw.tile([128, (TILE_E // 128) * nh], f32, tag="score", bufs=2)
                        nc.vector.tensor_reduce(score[:, 0:nsc], prod[:, 0:nsc, :], axis=X, op=ADD)
                        wt = sbw.tile([128, (TILE_E // 128) * nh], bf16, tag="wt", bufs=2)
                        nc.scalar.activation(wt[:, 0:nsc], score[:, 0:nsc], AF.Exp)
                        wg = sbw.tile([128, TILE_E // 128, numc], bf16, tag="wg", bufs=2)
                        w_view = wt[:, 0:nsc].rearrange("p (s h) -> p s h", s=nsub).unsqueeze(-1) \
                                             .broadcast_to([128, nsub, nh, hc])
                        nc.vector.tensor_mul(wg[:, 0:nsub, 0:ch].rearrange("p s (h c) -> p s h c", h=nh),
                                             xl_g[:, 0:nsub, 0:ch].rearrange("p s (h c) -> p s h c", h=nh),
                                             w_view)
                        nc.vector.tensor_copy(wg[:, 0:nsub, ch:numc],
                                              wt[:, 0:nsc].rearrange("p (s h) -> p s h", s=nsub))
                        for s in range(nsub):
                            nc.tensor.matmul(num_ps[:], oh_t[:, s, :], wg[:, s, :],
                                             start=(t == 0 and s == 0),
                                             stop=(t == ntiles - 1 and s == nsub - 1),
                                             skip_group_check=True)
                        off += ts
                    finalize(b, rows, num_ps)

            # ---------------- per-layer finalize ----------------
            def make_helu_finalize(h_elu):
                def fin(b, rows, num_ps):
                    recip = sbw.tile([128, HEADS], f32, tag="recip", bufs=2)
                    nc.vector.reciprocal(recip[:rows], num_ps[:rows, 256:260])
                    r_t = sbw.tile([128, 256], f32, tag="relu_t", bufs=2)
                    m_t = sbw.tile([128, 256], f32, tag="min_t", bufs=2)
                    for h in range(HEADS):
                        sl = slice(h * 64, (h + 1) * 64)
                        rb = recip[:rows, h:h + 1].broadcast_to([rows, 64])
                        nc.vector.scalar_tensor_tensor(r_t[:rows, sl], num_ps[:rows, sl],
                                                       0.0, rb, MAXOP, MULT)
                        nc.vector.scalar_tensor_tensor(m_t[:rows, sl], num_ps[:rows, sl],
                                                       0.0, rb, MINOP, MULT)
                    e_t = sbw.tile([128, 256], f32, tag="exp_t", bufs=2)
                    nc.scalar.activation(e_t[:rows], m_t[:rows], AF.Exp)
                    nc.vector.tensor_scalar(e_t[:rows], e_t[:rows], -1.0, None, ADD)
                    nc.vector.tensor_add(h_elu[:rows, b, :], r_t[:rows], e_t[:rows])
                return fin

            def mulv_finalize(b, rows, num_ps):
                recip = sbw.tile([128, 2], f32, tag="recip", bufs=2)
                nc.vector.reciprocal(recip[:rows], num_ps[:rows, 64:66])
                mu_sb = sbw.tile([128, LAT], f32, tag="mu_sb", bufs=2)
                lv_sb = sbw.tile([128, LAT], f32, tag="lv_sb", bufs=2)
                nc.vector.tensor_tensor(mu_sb[:rows], num_ps[:rows, 0:32],
                                        recip[:rows, 0:1].broadcast_to([rows, 32]), MULT)
                nc.vector.tensor_tensor(lv_sb[:rows], num_ps[:rows, 32:64],
                                        recip[:rows, 1:2].broadcast_to([rows, 32]), MULT)
                nc.sync.dma_start(mu_out[b * 128:b * 128 + rows, :], mu_sb[:rows])
                nc.sync.dma_start(lv_out[b * 128:b * 128 + rows, :], lv_sb[:rows])
                eps_sb = sbw.tile([128, LAT], f32, tag="eps_sb", bufs=2)
                nc.sync.dma_start(eps_sb[:rows], eps_in[b * 128:b * 128 + rows, :])
                t_e = sbw.tile([128, LAT], f32, tag="texp", bufs=2)
                nc.scalar.activation(t_e[:rows], lv_sb[:rows], AF.Exp, bias=0.0, scale=0.5)
                z_sb = sbw.tile([128, LAT], f32, tag="z_sb", bufs=2)
                nc.vector.tensor_mul(z_sb[:rows], t_e[:rows], eps_sb[:rows])
                nc.vector.tensor_add(z_sb[:rows], z_sb[:rows], mu_sb[:rows])
                zbf = sbw.tile([128, LAT], bf16, tag="zbf", bufs=2)
                nc.vector.tensor_copy(zbf[:rows], z_sb[:rows])
                nc.sync.dma_start(agz_in[b * 128:b * 128 + rows, :], zbf[:rows])
                zT_ps = psp.tile([32, 128], bf16, tag="shared")
                nc.tensor.transpose(zT_ps[:, 0:rows], zbf[0:rows, :], ident_bf[0:rows, 0:rows])
                nc.scalar.copy(zlocT[:, b * 128:b * 128 + rows], zT_ps[:, 0:rows])

            # ---------------- transposes h_elu -> hT ----------------
            def transpose_h(h_elu, hT):
                for nt in range(NBLK):
                    rows = ROWS[nt]
                    for k in range(2):
                        tps = psp.tile([128, 128], f32, tag="shared")
                        nc.tensor.transpose(tps[:], h_elu[:, nt, k * 128:(k + 1) * 128],
                                            ident_f32[:])
                        nc.scalar.copy(hT[:, k, nt * 128:nt * 128 + rows], tps[:, 0:rows])

            # ================ the network ================
            hT0 = persist.tile([128, 2, SHARD], f32, tag="hT0")
            nc.sync.dma_start(hT0[:], xT_in[:].rearrange("(k p) n -> p k n", p=128))

            xr0 = persist.tile([128, NBLK, 256], bf16, tag="xr01")
            nc.gpsimd.memset(xr0[:], 0.0)
            node_phase(hT0, wl0, wr0, 256, 256, ag0_in, xr0, pad128=False)
            nc.gpsimd.collective_compute("AllGather", mybir.AluOpType.bypass, RG,
                                         ins=[ag0_in[:]], outs=[tab0[:]])
            h_elu0 = persist.tile([128, NBLK, 256], f32, tag="helu0")
            nc.gpsimd.memset(h_elu0[:], 0.0)
            edge_phase(tab0, 256, 256, HEADS, 64, att0_sb, xr0, make_helu_finalize(h_elu0))

            hT1 = persist.tile([128, 2, SHARD], f32, tag="hT1")
            transpose_h(h_elu0, hT1)
            xr1 = persist.tile([128, NBLK, 256], bf16, tag="xr01")
            nc.gpsimd.memset(xr1[:], 0.0)
            node_phase(hT1, wl1, wr1, 256, 256, ag1_in, xr1, pad128=False)
            nc.gpsimd.collective_compute("AllGather", mybir.AluOpType.bypass, RG,
                                         ins=[ag1_in[:]], outs=[tab1[:]])
            h_elu1 = persist.tile([128, NBLK, 256], f32, tag="helu0")
            nc.gpsimd.memset(h_elu1[:], 0.0)
            edge_phase(tab1, 256, 256, HEADS, 64, att1_sb, xr1, make_helu_finalize(h_elu1))

            hT2 = persist.tile([128, 2, SHARD], f32, tag="hT0")
            transpose_h(h_elu1, hT2)
            xrm = persist.tile([128, NBLK, 64], bf16, tag="xrm")
            nc.gpsimd.memset(xrm[:], 0.0)
            node_phase(hT2, wml, wmr, 64, 64, agm_in, xrm, pad128=True)
            nc.gpsimd.collective_compute("AllGather", mybir.AluOpType.bypass, RG,
                                         ins=[agm_in[:]], outs=[tabm[:]])
            zlocT = persist.tile([32, 1280], bf16, tag="zlocT")
            edge_phase(tabm, 128, 64, 2, 32, attm_sb, xrm, mulv_finalize)

            nc.gpsimd.collective_compute("AllGather", mybir.AluOpType.bypass, RG,
                                         ins=[agz_in[:]], outs=[tabz[:]])

            # ---------------- adj = z @ z^T ----------------
            zt_sb = persist.tile([128, 79, LAT], bf16, tag="zt")
            nc.sync.dma_start(zt_sb[:, 0:78, :],
                              tabz[0:9984, :].rearrange("(t p) c -> p t c", p=128))
            nc.sync.dma_start(zt_sb[0:16, 78, :], tabz[9984:10000, :])
            zallT = persist.tile([32, 10112], bf16, tag="zallT")
            for t in range(79):
                rt = 128 if t < 78 else 16
                zps = psp.tile([32, 128], bf16, tag="shared")
                nc.tensor.transpose(zps[:, 0:rt], zt_sb[0:rt, t, :], ident_bf[0:rt, 0:rt])
                nc.scalar.copy(zallT[:, t * 128:t * 128 + rt], zps[:, 0:rt])

            NCHUNK = [512] * 19 + [272]
            for mt in range(NBLK):
                rows = ROWS[mt]
                for ck in range(20):
                    c0 = ck * 512
                    cw = NCHUNK[ck]
                    aps = psp.tile([128, 512], f32, tag="shared")
                    nc.tensor.matmul(aps[:rows, 0:cw], zlocT[:, mt * 128:mt * 128 + rows],
                                     zallT[:, c0:c0 + cw], start=True, stop=True)
                    ast = sbw.tile([128, 512], f32, tag="astage", bufs=4)
                    if ck % 2 == 0:
                        nc.scalar.copy(ast[:rows, 0:cw], aps[:rows, 0:cw])
                    else:
                        nc.vector.tensor_copy(ast[:rows, 0:cw], aps[:rows, 0:cw])
                    nc.sync.dma_start(adj_out[mt * 128:mt * 128 + rows, c0:c0 + cw],
                                      ast[:rows, 0:cw])

    nc.compile()
    return nc


_CACHE = {}


def kernel(**inputs):
    from concourse.bass_utils import run_bass_kernel_spmd

    x = np.asarray(inputs["x"], np.float32)
    eps = np.asarray(inputs["eps"], np.float32)
    plan, Ep, offs, idx_arrs, d_arrs, dT_arrs = _prep_edges(inputs["edge_index"])

    for k in inputs:
        if k.startswith(("bl_", "br_", "bo_")):
            assert not np.any(np.asarray(inputs[k])), f"nonzero bias {k} unsupported"

    key = tuple(tuple(p) for p in plan)
    if key not in _CACHE:
        _CACHE[key] = _build(plan, Ep, offs)
    nc = _CACHE[key]

    att0 = np.tile(np.asarray(inputs["att_l0"], np.float32).reshape(1, 256), (128, 8)).astype(bf)
    att1 = np.tile(np.asarray(inputs["att_l1"], np.float32).reshape(1, 256), (128, 8)).astype(bf)
    attm_row = np.concatenate([np.asarray(inputs["att_mu"], np.float32).reshape(32),
                               np.asarray(inputs["att_lv"], np.float32).reshape(32)])
    attm = np.tile(attm_row.reshape(1, 64), (128, 8)).astype(bf)
    wml = np.concatenate([np.asarray(inputs["Wl_mu"], np.float32),
                          np.asarray(inputs["Wl_lv"], np.float32)], axis=1)
    wmr = np.concatenate([np.asarray(inputs["Wr_mu"], np.float32),
                          np.asarray(inputs["Wr_lv"], np.float32)], axis=1)

    shared = {
        "wl0": np.ascontiguousarray(inputs["Wl_l0"], np.float32),
        "wr0": np.ascontiguousarray(inputs["Wr_l0"], np.float32),
        "wl1": np.ascontiguousarray(inputs["Wl_l1"], np.float32),
        "wr1": np.ascontiguousarray(inputs["Wr_l1"], np.float32),
        "wml": np.ascontiguousarray(wml),
        "wmr": np.ascontiguousarray(wmr),
        "att0": att0, "att1": att1, "attm": attm,
    }
    in_maps = []
    for r in range(NCORES):
        m = dict(shared)
        m["xT"] = np.ascontiguousarray(x[r * SHARD:(r + 1) * SHARD].T)
        m["eps"] = np.ascontiguousarray(eps[r * SHARD:(r + 1) * SHARD])
        m["idx"] = idx_arrs[r]
        m["dv"] = d_arrs[r]
        m["dT"] = dT_arrs[r]
        in_maps.append(m)

    res = run_bass_kernel_spmd(nc, in_maps, list(range(NCORES)))
    adj = np.concatenate([res.results[r]["adj"] for r in range(NCORES)], axis=0)
    mu = np.concatenate([res.results[r]["mu"] for r in range(NCORES)], axis=0)
    lv = np.concatenate([res.results[r]["lv"] for r in range(NCORES)], axis=0)
    return adj, mu, lv
